# revision 21
# baseline (speedup 1.0000x reference)
"""Multi-head causal attention on 8 Trainium2 NeuronCores.

Sharding: core c handles batch b = c // 2 and head-group g = c % 2
(8 of 16 heads, i.e. 512 of 1024 projection columns).  QKV projections
and attention run per-core in bf16; the two cores of a batch pair
exchange their attention outputs with a per-chunk AllGather (bf16,
overlapped with the next chunk's compute) and each core then computes
the output projection over the full 1024 attention features for its
half of the output columns — no end-of-kernel reduce, no final copy.

Everything on-device is computed in a transposed layout (seq on the
free dim) so no PE transposes are needed anywhere:
  xT [D, L] (host-pre-transposed bf16) -> qT/kT [512, L] -> S^T [keys, q]
  -> P^T = exp(S^T) (bf16) -> attn^T = (v|ones)^T @ P^T (Z in row 64)
  -> AllGather attn^T pair-wise -> out^T = Wo^T @ attn_full^T.

The emission order software-pipelines the attention at matmul
granularity: within a head, S(j) matmuls interleave with AV(j-3) so the
scalar-engine exp of tile j streams behind the S matmuls while the PE
keeps busy, and next-chunk projection / prev-chunk output-projection
matmul groups are spliced between heads so the PE never starves on the
exp chain.  Causal trimming skips the masked-out query columns of
diagonal key tiles in the S, exp and AV stages.
"""

import sys, types

sys.path.insert(0, "/opt/trn_rl_repo")

# antenv.axon_hooks is missing in this image; inject it so trace=True can
# reach the NTFF profiling hook (used by test.py, off by default).
if "antenv.axon_hooks" not in sys.modules:
    _hook_mod = types.ModuleType("antenv.axon_hooks")
    _hook_mod._hook = None
    def _set_hook(h):
        _hook_mod._hook = h
    def _get_hook():
        return _hook_mod._hook
    _hook_mod.set_axon_ntff_profile_hook = _set_hook
    _hook_mod.get_axon_ntff_profile_hook = _get_hook
    sys.modules["antenv.axon_hooks"] = _hook_mod
    try:
        import antenv
        antenv.axon_hooks = _hook_mod
        from trn_agent_boot.trn_boot import _ntff_profile_via_ctypes
        _set_hook(_ntff_profile_via_ctypes("/opt/axon/libaxon_pjrt.so"))
    except Exception:
        pass

import numpy as np
import ml_dtypes
import concourse.bass as bass
import concourse.mybir as mybir
import concourse.tile as tile
from concourse import bacc
from concourse.bass_utils import run_bass_kernel_spmd

B, L, D, H = 4, 2048, 1024, 16
DH = 64
N_CORES = 8
NH = 8          # heads per core
HC = NH * DH    # 512 projection cols per core
QC = 512        # q-chunk
KT = 128        # k-tile
P = 128
NQC = L // QC   # 4
NKT = L // KT   # 16
NDS = D // P    # 8 contraction tiles for projections
NO = (D // 2) // P  # 4 output-column tiles per core

F32 = mybir.dt.float32
BF16 = mybir.dt.bfloat16

PAIRS = [[0, 1], [2, 3], [4, 5], [6, 7]]

TRACE = False
LAST_EXEC_NS = None
_NC = None


def build_nc():
    nc = bacc.Bacc()

    xT = nc.declare_dram_parameter("xT", [D, L], BF16, isOutput=False)
    wq = nc.declare_dram_parameter("wq", [D, HC], BF16, isOutput=False)
    wk = nc.declare_dram_parameter("wk", [D, HC], BF16, isOutput=False)
    wv = nc.declare_dram_parameter("wv", [D, HC], BF16, isOutput=False)
    wo = nc.declare_dram_parameter("wo", [D, D // 2], BF16, isOutput=False)
    bqb = nc.declare_dram_parameter("bqb", [P, 4 * QC], F32, isOutput=False)
    bkb = nc.declare_dram_parameter("bkb", [P, 4 * QC], F32, isOutput=False)
    bvb = nc.declare_dram_parameter("bvb", [P, HC], F32, isOutput=False)
    bo = nc.declare_dram_parameter("bo", [P, NO], F32, isOutput=False)
    m01 = nc.declare_dram_parameter("m01", [P, 4 * QC], BF16, isOutput=False)
    outTh = nc.declare_dram_parameter("outTh", [D // 2, L], F32, isOutput=True)

    scale = 1.0 / np.sqrt(np.float32(DH))

    from collections import deque
    from contextlib import ExitStack
    with nc.allow_low_precision(reason="bf16 matmuls throughout by design"), \
         tile.TileContext(nc) as tc, ExitStack() as ctx:
        consts = ctx.enter_context(tc.tile_pool(name="consts", bufs=1))
        wpool = ctx.enter_context(tc.tile_pool(name="wpool", bufs=1))
        kvres = ctx.enter_context(tc.tile_pool(name="kvres", bufs=1))
        xtp = ctx.enter_context(tc.tile_pool(name="xtp", bufs=4))
        qtp = ctx.enter_context(tc.tile_pool(name="qtp", bufs=8))
        ptp = ctx.enter_context(tc.tile_pool(name="ptp", bufs=36))
        anp = ctx.enter_context(tc.tile_pool(name="anp", bufs=4))
        agsp = ctx.enter_context(tc.tile_pool(name="agsp", bufs=24))
        otp = ctx.enter_context(tc.tile_pool(name="otp", bufs=3))
        zrp = ctx.enter_context(tc.tile_pool(name="zrp", bufs=2))
        zsp = ctx.enter_context(tc.tile_pool(name="zsp", bufs=4))
        bzsb = ctx.enter_context(tc.tile_pool(name="bzsb", bufs=4))
        zdp = ctx.enter_context(tc.tile_pool(name="zdp", bufs=4, space="DRAM"))
        aginp = ctx.enter_context(tc.tile_pool(name="aginp", bufs=4, space="DRAM"))
        agoutp = ctx.enter_context(tc.tile_pool(name="agoutp", bufs=4, space="DRAM"))
        scratch = ctx.enter_context(tc.tile_pool(name="scratch", bufs=2, space="PSUM"))
        stp = ctx.enter_context(tc.tile_pool(name="stp", bufs=4, space="PSUM"))
        accp = ctx.enter_context(tc.tile_pool(name="accp", bufs=2, space="PSUM"))

        # ---- per-chunk state ----
        xT_t = {}
        qT_t = {}
        agsb_t = {}

        def emit_xT(c):
            for g in range(2):
                xt = xtp.tile([P, 4, QC], BF16, tag="xT", name="xT4")
                nc.sync.dma_start(
                    out=xt,
                    in_=xT[4 * g * P:4 * (g + 1) * P,
                           c * QC:(c + 1) * QC].rearrange(
                               "(d p) q -> p d q", p=P))
                for i in range(4):
                    xT_t[(4 * g + i, c)] = xt[:, i, :]

        # ---- first-chunk input first, then constants and weights ----
        emit_xT(0)
        wq_all = wpool.tile([P, NDS, HC], BF16, tag="wqa", name="wqa")
        for g in range(2):
            nc.sync.dma_start(
                out=wq_all[:, 4 * g:4 * (g + 1), :],
                in_=wq[4 * g * P:4 * (g + 1) * P, :].rearrange(
                    "(d p) c -> p d c", p=P))
        wq_sb = [wq_all[:, ds, :] for ds in range(NDS)]
        wk_all = wpool.tile([P, NDS, HC], BF16, tag="wka", name="wka")
        for g in range(2):
            nc.sync.dma_start(
                out=wk_all[:, 4 * g:4 * (g + 1), :],
                in_=wk[4 * g * P:4 * (g + 1) * P, :].rearrange(
                    "(d p) c -> p d c", p=P))
        wk_sb = [wk_all[:, ds, :] for ds in range(NDS)]
        bqb_sb = consts.tile([P, 4, QC], F32, tag="bqb")
        nc.sync.dma_start(out=bqb_sb, in_=bqb[:, :].rearrange("p (t q) -> p t q", t=4))
        bkb_sb = consts.tile([P, 4, QC], F32, tag="bkb")
        nc.sync.dma_start(out=bkb_sb, in_=bkb[:, :].rearrange("p (t q) -> p t q", t=4))
        wv_all = wpool.tile([P, NDS, HC], BF16, tag="wva", name="wva")
        for g in range(2):
            nc.sync.dma_start(
                out=wv_all[:, 4 * g:4 * (g + 1), :],
                in_=wv[4 * g * P:4 * (g + 1) * P, :].rearrange(
                    "(d p) c -> p d c", p=P))
        wv_sb = [wv_all[:, ds, :] for ds in range(NDS)]
        bvb_sb = consts.tile([P, NH, DH], F32, tag="bvb")
        nc.sync.dma_start(out=bvb_sb, in_=bvb[:, :].rearrange("p (h d) -> p h d", h=NH))
        m01_sb = consts.tile([P, 4, QC], BF16, tag="m01")
        nc.sync.dma_start(out=m01_sb, in_=m01[:, :].rearrange("p (m q) -> p m q", m=4))
        wo_all = wpool.tile([P, NDS, D // 2], BF16, tag="woa", name="woa")
        for g in range(2):
            nc.sync.dma_start(
                out=wo_all[:, 4 * g:4 * (g + 1), :],
                in_=wo[4 * g * P:4 * (g + 1) * P, :].rearrange(
                    "(d p) c -> p d c", p=P))
        wo_sb = [wo_all[:, k, :] for k in range(NDS)]
        bo_sb = consts.tile([P, NO], F32, tag="bo")
        nc.sync.dma_start(out=bo_sb, in_=bo[:, :])

        # ---- resident kT and v ----
        kT_sb = [kvres.tile([P, L], BF16, tag=f"kT{t}", name=f"kT{t}") for t in range(HC // P)]
        # v: per key-tile [128, NH, 65] bf16; cols 0..63 = v, col 64 = ones
        # (the ones column makes the AV matmul emit softmax Z in row 64)
        v_sb = [kvres.tile([P, NH, 65], BF16, tag=f"v{kt}", name=f"v{kt}") for kt in range(NKT)]
        for kt in range(NKT):
            nc.vector.memset(v_sb[kt][:, :, DH:DH + 1], 1.0)

        # ---- filler generators: one matmul per yield ----
        def gen_q(t, c):
            pq = scratch.tile([P, QC], F32, tag="pacc")
            for ds in range(NDS):
                nc.tensor.matmul(
                    pq, wq_sb[ds][:, t * P:(t + 1) * P], xT_t[(ds, c)],
                    start=(ds == 0), stop=(ds == NDS - 1))
                yield
            qt = qtp.tile([P, QC], BF16, tag="qT")
            nc.vector.tensor_add(qt, pq, bqb_sb[:, t, :])
            qT_t[(t, c)] = qt

        def gen_k(t, c):
            pk = scratch.tile([P, QC], F32, tag="pacc")
            for ds in range(NDS):
                nc.tensor.matmul(
                    pk, wk_sb[ds][:, t * P:(t + 1) * P], xT_t[(ds, c)],
                    start=(ds == 0), stop=(ds == NDS - 1))
                yield
            nc.vector.tensor_add(
                kT_sb[t][:, c * QC:(c + 1) * QC], pk, bkb_sb[:, t, :])

        def gen_v(sub, c):
            kt = c * (QC // P) + sub
            pv = scratch.tile([P, HC], F32, tag="pacc")
            for ds in range(NDS):
                nc.tensor.matmul(
                    pv, xT_t[(ds, c)][:, sub * P:(sub + 1) * P], wv_sb[ds],
                    start=(ds == 0), stop=(ds == NDS - 1))
                yield
            nc.vector.tensor_add(
                v_sb[kt][:, :, 0:DH],
                pv[:].rearrange("p (h d) -> p h d", h=NH),
                bvb_sb)

        def gen_oproj(o, c):
            ks = ([0, 1, 4, 5, 2, 6, 3, 7] if c == NQC - 1
                  else [0, 1, 4, 5, 2, 3, 6, 7])
            po = scratch.tile([P, QC], F32, tag="pacc")
            for i, k in enumerate(ks):
                nc.tensor.matmul(
                    po, wo_sb[k][:, o * P:(o + 1) * P], agsb_t[(k, c)],
                    start=(i == 0), stop=(i == NDS - 1))
                yield
            ot = otp.tile([P, QC], F32, tag="ot")
            nc.scalar.activation(
                out=ot, in_=po,
                func=mybir.ActivationFunctionType.Identity,
                bias=bo_sb[:, o:o + 1], scale=1.0)
            nc.sync.dma_start(
                out=outTh[o * P:(o + 1) * P, c * QC:(c + 1) * QC], in_=ot)

        def proj_gens(c):
            for t in range(HC // P):
                yield gen_q(t, c)
                yield gen_k(t, c)
            for sub in range(QC // P):
                yield gen_v(sub, c)

        def make_filler(gens):
            q = deque(gens)
            def pop():
                while q:
                    try:
                        next(q[0])
                        return True
                    except StopIteration:
                        q.popleft()
                return False
            return pop

        # ---- attention for one head, S/AV interleaved at tile grain ----
        def attn_pair(t, c, ag_in_t, row_base, fill_pop, n_pops):
            # Both heads (par 0/1) of t-tile t are processed together: their
            # S matmuls contract over disjoint 64-partition halves, so bass
            # auto-assigns PE tile positions (0,0)/(64,0) and adjacent
            # emission lets the two run concurrently in the array.
            njt = 4 * c + 4
            pts = {0: [], 1: []}
            accs = {}

            def S_micro(par, j):
                m = j - 4 * c
                lo = P * m if m > 0 else 0
                st = stp.tile([P, QC], F32, tag="st")
                nc.tensor.matmul(
                    st[:, lo:QC],
                    kT_sb[t][par * DH:(par + 1) * DH, j * KT:(j + 1) * KT],
                    qT_t[(t, c)][par * DH:(par + 1) * DH, lo:QC],
                    start=True, stop=True)
                pt = ptp.tile([P, QC], BF16, tag="pt")
                nc.scalar.activation(
                    out=pt[:, lo:QC], in_=st[:, lo:QC],
                    func=mybir.ActivationFunctionType.Exp,
                    scale=float(scale))
                if m >= 0:
                    nc.vector.tensor_mul(
                        pt[:, lo:QC], pt[:, lo:QC], m01_sb[:, m, lo:QC])
                pts[par].append((pt, lo))

            def AV_micro(par, j):
                pt, lo = pts[par][j]
                if j == 0:
                    accs[par] = accp.tile([P, QC], F32, tag="acc", name="acc")
                nc.tensor.matmul(
                    accs[par][0:DH + 1, lo:QC],
                    v_sb[j][:, 2 * t + par, :],
                    pt[:, lo:QC],
                    start=(j == 0), stop=(j == njt - 1))

            pops_done = 0
            reserve = min(10, n_pops // 2 + 1)
            for j in range(njt):
                S_micro(0, j)
                S_micro(1, j)
                if j >= 5:
                    AV_micro(0, j - 5)
                    AV_micro(1, j - 5)
                for _ in range(2):
                    if pops_done < n_pops - reserve and fill_pop():
                        pops_done += 1
            for j in range(max(njt - 5, 0), njt):
                AV_micro(0, j)
                AV_micro(1, j)
                for _ in range(2):
                    if pops_done < n_pops and fill_pop():
                        pops_done += 1
            while pops_done < n_pops and fill_pop():
                pops_done += 1

            # normalization: copy each AV accumulator (rows 0..63 = attn,
            # row 64 = Z) out of PSUM right away so the bank frees, then the
            # 1/Z broadcast chain runs from SBUF off the critical path.
            finishes = []
            for par in (0, 1):
                acc = accs[par]
                zsb = zsp.tile([P, QC], F32, tag="zsb", name="zsb")
                nc.scalar.activation(
                    out=zsb[0:DH + 1, :], in_=acc[0:DH + 1, :],
                    func=mybir.ActivationFunctionType.Identity, scale=1.0)
                # reciprocal_approx_fast mis-executes on partition-offset
                # APs, so run it over partitions 0..64 (offset 0); rows
                # 0..63 hold garbage reciprocals that are never read.
                zrec = zrp.tile([P, QC], F32, tag="zrec")
                nc.vector.reciprocal_approx_fast(
                    out=zrec[0:DH + 1, :], in_=zsb[0:DH + 1, :])
                zrow = zdp.tile([1, QC], F32, tag="zd", name="zrow")
                nc.sync.dma_start(out=zrow, in_=zrec[DH:DH + 1, :])
                bzs = bzsb.tile([DH, QC], F32, tag="bzs")
                nc.sync.dma_start(out=bzs[0:DH, :].unsqueeze(1),
                                  in_=zrow.partition_broadcast(DH))

                def finish(par=par, zsb=zsb, bzs=bzs):
                    an = anp.tile([DH, QC], BF16, tag="an", name="an")
                    nc.vector.tensor_mul(an, zsb[0:DH, :], bzs)
                    row = row_base + par * DH
                    nc.sync.dma_start(out=ag_in_t[row:row + DH, :], in_=an)
                finishes.append(finish)
            return finishes

        def ag_emit(c, ag_in_t, nt, tag):
            # the piece covers nt consecutive t-tiles; the AllGather output
            # concatenates the even core's nt*128 rows then the odd core's,
            # so piece rows map to global k-tiles {ts...} and {4+ts...}.
            ag_out_t = agoutp.tile([2 * nt * P, QC], BF16, tag=tag, name=tag)
            nc.gpsimd.collective_compute(
                "AllGather", mybir.AluOpType.bypass,
                replica_groups=PAIRS,
                ins=[ag_in_t.opt()], outs=[ag_out_t.opt()],
            )
            return ag_out_t

        def agsb_read(c, ag_out_t, t_lo, nt, eng):
            ks = [t_lo + i for i in range(nt)] + [4 + t_lo + i for i in range(nt)]
            for i, k in enumerate(ks):
                ag = agsp.tile([P, QC], BF16, tag="agsb")
                eng.dma_start(out=ag, in_=ag_out_t[i * P:(i + 1) * P, :])
                agsb_t[(k, c)] = ag

        # ---- chunk 0 projections up-front ----
        p0 = make_filler(proj_gens(0))
        while p0():
            pass

        # ---- main pipeline over chunks ----
        for c in range(NQC):
            if c + 1 < NQC:
                emit_xT(c + 1)
            gens = list(proj_gens(c + 1)) if c + 1 < NQC else []
            if c >= 2:
                gens += [gen_oproj(o, c - 2) for o in range(NO)]
            if c == NQC - 1:
                gens += [gen_oproj(o, c - 1) for o in range(NO)]
            n_micros = 8 * len(gens)
            fill_pop = make_filler(gens)
            n_pops = (n_micros + 3) // 4 if n_micros else 0
            last = c == NQC - 1
            ag_in_a = aginp.tile([HC // 2, QC], BF16, tag="agin", name="agin_a")
            if last:
                ag_in_b = aginp.tile([P, QC], BF16, tag="aginq", name="agin_t2")
                ag_in_d = aginp.tile([P, QC], BF16, tag="aginq", name="agin_t3")
            else:
                ag_in_b = aginp.tile([HC // 2, QC], BF16, tag="agin", name="agin_b")
            pending = []
            ag_out_a = ag_out_b = None
            for t in range(4):
                if t < 2:
                    tgt, base = ag_in_a, t * P
                elif last:
                    tgt, base = (ag_in_b, 0) if t == 2 else (ag_in_d, 0)
                else:
                    tgt, base = ag_in_b, (t - 2) * P
                nxt = attn_pair(t, c, tgt, base, fill_pop, n_pops)
                for f in pending:
                    f()
                pending = nxt
                if t == 2:
                    ag_out_a = ag_emit(c, ag_in_a, 2, "agout")
                    if not last:
                        # gpsimd SWDGE path: slow but does not head-of-line
                        # block the sync HWDGE rings carrying the per-head
                        # zrow/bzs/an traffic of the chunk in flight
                        agsb_read(c, ag_out_a, 0, 2, nc.gpsimd)
                if last and t == 3:
                    # t=2 piece complete after its deferred finishes above
                    ag_out_b = ag_emit(c, ag_in_b, 1, "agoutq")
            for f in pending:
                f()
            while fill_pop():
                pass
            if last:
                # bounce reads land on the now-idle sync rings after every
                # per-head DMA of this chunk, so they block nothing
                agsb_read(c, ag_out_a, 0, 2, nc.sync)
                agsb_read(c, ag_out_b, 2, 1, nc.sync)
                # emit the first output-projection pair's matmuls over the
                # already-exchanged k-tiles before the final AllGather
                g0, g1 = gen_oproj(0, c), gen_oproj(1, c)
                for _ in range(6):
                    next(g0)
                for _ in range(6):
                    next(g1)
                ag_out_d = ag_emit(c, ag_in_d, 1, "agoutq")
                # separate wait domain (ACT HWDGE queue) so the earlier
                # k-tiles' matmul waits are not merged behind these reads
                agsb_read(c, ag_out_d, 3, 1, nc.scalar)
                for g in (g0, g1):
                    try:
                        while True:
                            next(g)
                    except StopIteration:
                        pass
                g2, g3 = gen_oproj(2, c), gen_oproj(3, c)
                for _ in range(6):
                    next(g2)
                for _ in range(6):
                    next(g3)
                for g in (g2, g3):
                    try:
                        while True:
                            next(g)
                    except StopIteration:
                        pass
            else:
                ag_out_b = ag_emit(c, ag_in_b, 2, "agout")
                agsb_read(c, ag_out_b, 2, 2, nc.gpsimd)

    nc.compile()
    return nc


def _make_in_maps(x, Wq, bq, Wk, bk, Wv, bv, Wo, bo, mask):
    ref = np.tril(np.ones((L, L), dtype=np.int32))[None, None]
    assert np.array_equal(np.asarray(mask), ref), "mask must be causal"

    # m01 patterns for the 4 diagonal k-tiles of a q-chunk:
    # pattern_m[p, f] = 1 if p <= f - 128*m
    pf = np.arange(QC)[None, :] - np.arange(P)[:, None]
    m01 = np.concatenate(
        [(pf >= P * m).astype(np.float32) for m in range(4)], axis=1)

    def bcast_bias(b_slice):
        # [512] -> [128 partitions, 4 t-tiles, 512 cols] broadcast over cols
        return np.ascontiguousarray(
            np.broadcast_to(
                np.asarray(b_slice).reshape(4, P, 1).transpose(1, 0, 2),
                (P, 4, QC)).reshape(P, 4 * QC)).astype(np.float32)

    in_maps = []
    for c in range(N_CORES):
        b, g = c // 2, c % 2
        cols = slice(HC * g, HC * g + HC)
        half = slice((D // 2) * g, (D // 2) * g + D // 2)
        in_maps.append({
            "xT": np.ascontiguousarray(np.asarray(x[b]).T).astype(ml_dtypes.bfloat16),
            "wq": np.ascontiguousarray(np.asarray(Wq)[:, cols]).astype(ml_dtypes.bfloat16),
            "wk": np.ascontiguousarray(np.asarray(Wk)[:, cols]).astype(ml_dtypes.bfloat16),
            "wv": np.ascontiguousarray(np.asarray(Wv)[:, cols]).astype(ml_dtypes.bfloat16),
            "wo": np.ascontiguousarray(np.asarray(Wo)[:, half]).astype(ml_dtypes.bfloat16),
            "bqb": bcast_bias(np.asarray(bq)[cols]),
            "bkb": bcast_bias(np.asarray(bk)[cols]),
            "bvb": np.ascontiguousarray(
                np.broadcast_to(np.asarray(bv)[cols], (P, HC))).astype(np.float32),
            "bo": np.ascontiguousarray(
                np.asarray(bo)[half].reshape(NO, P).T.astype(np.float32)),
            "m01": m01.astype(ml_dtypes.bfloat16),
        })
    return in_maps


def kernel(x, Wq, bq, Wk, bk, Wv, bv, Wo, bo, mask):
    global _NC, LAST_EXEC_NS
    if _NC is None:
        _NC = build_nc()
    in_maps = _make_in_maps(x, Wq, bq, Wk, bk, Wv, bv, Wo, bo, mask)
    r = run_bass_kernel_spmd(
        _NC, in_maps, core_ids=list(range(N_CORES)), trace=TRACE)
    LAST_EXEC_NS = r.exec_time_ns
    out = np.empty((B, L, D), dtype=np.float32)
    for b in range(B):
        outT = np.concatenate(
            [r.results[2 * b]["outTh"], r.results[2 * b + 1]["outTh"]], axis=0)
        out[b] = outT.T
    return out
